# revision 10
# baseline (speedup 1.0000x reference)
"""Tensor-parallel attention block (QKV + RoPE + causal attention + out-proj)
for 8 Trainium2 NeuronCores.

Sharding: heads (16) split across 8 cores, 2 heads/core. wq/wk/wv column-
sharded, wo row-sharded; each core computes a full-shape partial output and
the host sums the 8 partials.

Layout trick: everything on the PE array is a natural `lhsT.T @ rhs`:
  - host pre-transposes x -> xT [D, B*S] so projections need no transposes
  - q,k produced in [head_dim, seq] layout; scores computed TRANSPOSED
    ([k_seq, q_seq]) so softmax needs no on-chip transposes at all
  - softmax denominator: DVE accumulates exp'd prob chunks into an f32 SBUF
    tile, then ONE ones-matmul per q-window gives the partition sum (keeps
    the PE out of the per-chunk denominator business)
  - RoPE pair-halves are deinterleaved by permuting wq/wk rows on host;
    the half-swap needed by rotation is a chunked SBUF->SBUF DMA pipelined
    right behind each seq-group's projections
  - mask handled exactly as multiplicative exp(mask) tiles; all-zero tiles
    skip compute, all-one tiles skip the multiply (derived from the real
    mask values at build time, not assumed causal)
"""

import math
import os
import sys

import numpy as np
import ml_dtypes

sys.path.insert(0, "/opt/trn_rl_repo")

import concourse.bass as bass
import concourse.mybir as mybir
from concourse.tile import TileContext
from concourse.bass_utils import run_bass_kernel_spmd
from concourse.masks import make_identity

B, S, D, H = 2, 2048, 2048, 16
HD = D // H            # 128 head dim
NCORES = 8
HC = H // NCORES       # 2 heads per core
DHC = HC * HD          # 256
BS = B * S             # 4096
NDIN = D // 128        # 16 contraction chunks
W = 512                # attention q-window / matmul free size
NQW = S // W           # 4 q windows per batch
NKC = S // 128         # 16 k chunks per batch
SG = 1024              # qkv seq-group width
NSG = BS // SG         # 4
RSQRT_HD = 1.0 / math.sqrt(HD)
NWARM = 150            # PE warmup matmuls (keep HAM at 8/8 until data lands)

BF16 = mybir.dt.bfloat16
F32 = mybir.dt.float32
NPBF16 = ml_dtypes.bfloat16

SKIP, FREE, MASKED = 0, 1, 2

# stash of the last BassKernelResults for the test harness (exec_time_ns etc)
LAST_RUN = [None]
_PROGRAM_CACHE = {}


def _split_multi_waits(nc):
    """Walrus codegen allows only 1 embedded sync-wait per instruction (2 for
    EventSemaphore). Tile's sem-assignment can emit more; hoist the excess into
    standalone InstEventSemaphore waits on the same engine, just before."""
    n = 0
    for blk in nc.m.functions[0].blocks:
        out = []
        for inst in blk.instructions:
            si = getattr(inst, "sync_info", None)
            cap = 2 if isinstance(inst, mybir.InstEventSemaphore) else 1
            if si is not None and si.on_wait and len(si.on_wait) > cap:
                waits = list(si.on_wait)
                for w in waits[:-cap]:
                    n += 1
                    ev = mybir.InstEventSemaphore(
                        name=f"{inst.name}_xw{n}",
                        ins=[], outs=[],
                        sync_info=mybir.SyncInfo(on_wait=[w], on_update=[]))
                    ev.engine = inst.engine
                    out.append(ev)
                si.on_wait = waits[-cap:]
            out.append(inst)
        blk.instructions = out


def _build(cls_key):
    """Build the per-core Bass program. cls_key: tuple[NQW][NKC] of SKIP/FREE/MASKED."""
    cls = [list(row) for row in cls_key]
    nc = bass.Bass()

    xT = nc.declare_dram_parameter("xT", [D, BS], BF16, isOutput=False)
    # weights chunk-major: [128, NDIN, DHC] flattened, quad q holds di 4q..4q+3
    wqT = nc.declare_dram_parameter("wqT", [128, NDIN * DHC], BF16, isOutput=False)
    wkT = nc.declare_dram_parameter("wkT", [128, NDIN * DHC], BF16, isOutput=False)
    wvT = nc.declare_dram_parameter("wvT", [128, NDIN * DHC], BF16, isOutput=False)
    woT = nc.declare_dram_parameter("woT", [DHC, D], BF16, isOutput=False)
    trig = nc.declare_dram_parameter("trig", [128, 2 * S], BF16, isOutput=False)
    emaskT = nc.declare_dram_parameter("emaskT", [S, S], BF16, isOutput=False)
    out_d = nc.declare_dram_parameter("out", [BS, D], BF16, isOutput=True)

    with TileContext(nc) as tc:
        with (
            tc.tile_pool(name="consts", bufs=1) as consts,
            tc.tile_pool(name="xt", bufs=5) as xtp,
            tc.tile_pool(name="rsw", bufs=4) as rswp,
            tc.tile_pool(name="rm", bufs=4) as rmp,
            tc.tile_pool(name="vtmp", bufs=2) as vtp,
            tc.tile_pool(name="probs", bufs=5) as prp,
            tc.tile_pool(name="acc", bufs=3) as accp,
            tc.tile_pool(name="emask", bufs=6) as emp,
            tc.tile_pool(name="small", bufs=2) as smp,
            tc.tile_pool(name="outsb", bufs=3) as outp,
            tc.tile_pool(name="psQ", bufs=3, space="PSUM") as psQ,
            tc.tile_pool(name="psS", bufs=2, space="PSUM") as psS,
            tc.tile_pool(name="psC", bufs=2, space="PSUM") as psC,
        ):
            # persistent tiles
            q_sb = [consts.tile([128, BS], BF16, tag=f"q{h}", name=f"q{h}") for h in range(HC)]
            k_sb = [consts.tile([128, BS], BF16, tag=f"k{h}", name=f"k{h}") for h in range(HC)]
            a_sb = [consts.tile([128, BS], BF16, tag=f"a{h}", name=f"a{h}") for h in range(HC)]
            vT_sb = consts.tile([128, B * NKC * DHC], BF16, tag="vT", name="vT")
            ident = consts.tile([128, 128], BF16, tag="ident", name="ident")
            make_identity(nc, ident)
            ones = consts.tile([128, 128], BF16, tag="ones", name="ones")
            nc.vector.memset(ones, 1.0)

            # qkv weights: 4-chunk quads so the first matmul only waits on
            # 256 KB, not the full megabyte. wt[ti][q][:, j, :] = chunk 4q+j.
            wt = []
            for wi_, wparam in enumerate([wqT, wkT, wvT]):
                wv_ = wparam.rearrange("p (n m) -> p n m", n=NDIN)
                quads = []
                for qd in range(4):
                    t_ = consts.tile([128, 4, DHC], BF16, tag=f"w{wi_}_{qd}",
                                     name=f"w{wi_}_{qd}")
                    quads.append(t_)
                wt.append(quads)
            # DMA order: everything QKV needs for sg0 first, in consumption order
            nc.sync.dma_start(out=wt[0][0], in_=wqT.rearrange(
                "p (n m) -> p n m", n=NDIN)[:, 0:4, :])
            # warm the PE clock (HAM releases the 1.2GHz throttle after ~3.4us
            # of sustained activity) while the first DMAs are in flight
            wu = psC.tile([128, 128], F32, tag="dsum", bufs=1, name="warmup")
            for i in range(NWARM):
                nc.tensor.matmul(wu, lhsT=ones, rhs=ones, start=True, stop=True)

            def load_xt_group(sg, fine=False):
                xts = []
                for dj in range(4):
                    tb = xtp.tile([128, 4, SG], BF16, tag="xt", name=f"xt{sg}_{dj}")
                    src = xT[dj * 512:(dj + 1) * 512,
                             sg * SG:(sg + 1) * SG].rearrange("(n p) m -> p n m", p=128)
                    for k4 in range(4):
                        eng = nc.sync if (dj * 4 + k4) % 2 == 0 else nc.scalar
                        if fine and dj < 2:
                            eng.dma_start(out=tb[:, k4, 0:W], in_=src[:, k4, 0:W])
                            eng.dma_start(out=tb[:, k4, W:SG], in_=src[:, k4, W:SG])
                        else:
                            eng.dma_start(out=tb[:, k4, :], in_=src[:, k4, :])
                        xts.append(tb[:, k4, :])
                return xts

            xts_cur = load_xt_group(0, fine=True)
            for wi_ in range(3):
                for qd in range(4):
                    if wi_ == 0 and qd == 0:
                        continue
                    nc.gpsimd.dma_start(
                        out=wt[wi_][qd],
                        in_=[wqT, wkT, wvT][wi_].rearrange(
                            "p (n m) -> p n m", n=NDIN)[:, 4 * qd:4 * qd + 4, :])
            trig_sb = consts.tile([128, 2 * S], BF16, tag="trig", name="trig")
            woT_sb = [consts.tile([128, D], BF16, tag=f"wo{h}", name=f"wo{h}")
                      for h in range(HC)]

            def rope_chunk(tens, h, b, cc, width):
                """Rotate tens[h][:, cc:cc+width] in place (cc global col)."""
                src = tens[h]
                sp = cc - b * S  # position within the batch for trig lookup
                sw = rswp.tile([128, W], BF16, tag="rsw", name=f"sw{cc}_{h}")
                nc.gpsimd.dma_start(out=sw[0:64, :width], in_=src[64:128, cc:cc + width])
                nc.gpsimd.dma_start(out=sw[64:128, :width], in_=src[0:64, cc:cc + width])
                mcc = rmp.tile([128, W], BF16, tag="mcc", name=f"mcc{cc}_{h}")
                mss = rmp.tile([128, W], BF16, tag="mss", name=f"mss{cc}_{h}")
                nc.vector.tensor_mul(mcc[:, :width], src[:, cc:cc + width],
                                     trig_sb[:, sp:sp + width])
                nc.vector.tensor_mul(mss[:, :width], sw[:, :width],
                                     trig_sb[:, S + sp:S + sp + width])
                nc.vector.tensor_add(src[:, cc:cc + width], mcc[:, :width],
                                     mss[:, :width])

            def qkv_group(sg, ti, dh, xts):
                """One projection group: 32 matmuls -> 2 psum tiles -> copies."""
                ps = [psQ.tile([128, W], F32, tag="q", name=f"psA{sg}_{ti}_{dh}_{wi}")
                      for wi in range(2)]
                for di in range(NDIN):
                    for wi in range(2):
                        nc.tensor.matmul(
                            ps[wi], lhsT=wt[ti][di // 4][:, di % 4, dh * 128:(dh + 1) * 128],
                            rhs=xts[di][:, wi * W:(wi + 1) * W],
                            start=(di == 0), stop=(di == NDIN - 1))
                for wi in range(2):
                    c0 = sg * SG + wi * W
                    if ti < 2:
                        dst = (q_sb if ti == 0 else k_sb)[dh]
                        with tc.high_priority():
                            if wi == 0:
                                nc.scalar.copy(dst[:, c0:c0 + W], ps[wi])
                            else:
                                nc.vector.tensor_copy(dst[:, c0:c0 + W], ps[wi])
                    else:
                        vt = vtp.tile([128, W], BF16, tag="vtmp", name=f"vt{sg}_{dh}_{wi}")
                        with tc.high_priority():
                            nc.scalar.copy(vt, ps[wi])
                        for j in range(W // 128):
                            pt = psC.tile([128, 128], BF16, tag="att",
                                          name=f"pvt{sg}_{dh}_{wi}_{j}")
                            nc.tensor.transpose(pt, vt[:, j * 128:(j + 1) * 128], ident)
                            g = (c0 + j * 128) // 128
                            o0 = g * DHC + dh * 128
                            nc.scalar.copy(vT_sb[:, o0:o0 + 128], pt)
                # pipeline RoPE right behind each projection so batch-1
                # scores aren't gated on a late half-swap DMA
                if ti < 2 and dh == 1:
                    tens = q_sb if ti == 0 else k_sb
                    b = sg // 2
                    for dh_ in range(2):
                        for wi in range(2):
                            rope_chunk(tens, dh_, b, sg * SG + wi * W, W)

            def attn_window_em(b, qw):
                active = [c for c in range(NKC) if cls[qw][c][0] != SKIP]
                em_tiles = {}
                for c in active:
                    if cls[qw][c][0] == MASKED:
                        kind, off, tri = cls[qw][c]
                        if tri:
                            # only the 128-wide diagonal band is partial
                            hi = min(off + 128, W)
                            em = emp.tile([128, 128], BF16, tag="em", name=f"em{b}_{qw}_{c}")
                            nc.gpsimd.dma_start(
                                out=em[:, :hi - off],
                                in_=emaskT[c * 128:(c + 1) * 128,
                                           qw * W + off:qw * W + hi])
                        else:
                            em = emp.tile([128, W], BF16, tag="em", name=f"em{b}_{qw}_{c}")
                            nc.gpsimd.dma_start(
                                out=em,
                                in_=emaskT[c * 128:(c + 1) * 128, qw * W:(qw + 1) * W])
                        em_tiles[c] = em
                return active, em_tiles

            def attn_window(b, qw, h, active, em_tiles, pe_dsum=False):
                """Scores -> exp -> AV accumulate -> denominator -> normalize
                for one 512-wide q window of one head. Masked (diagonal) tiles
                only compute columns >= off: columns below the first unmasked
                one are exactly zero and contribute nothing downstream."""
                qc = b * S + qw * W
                att = psC.tile([128, W], F32, tag="att", name=f"att{b}_{h}_{qw}")
                dsm = psC.tile([128, W], F32, tag="dsum", bufs=1, name=f"dsm{b}_{h}_{qw}")
                acc = None
                if not pe_dsum:
                    acc = accp.tile([128, W], BF16, tag="acc", name=f"acc{b}_{h}_{qw}")
                n = len(active)
                for ci, c in enumerate(active):
                    kind, off, tri = cls[qw][c]
                    if kind != MASKED:
                        off = 0
                    wd = W - off
                    sp = psS.tile([128, W], F32, tag="s", name=f"sc{b}_{h}_{qw}_{c}")
                    kc = b * S + c * 128
                    nc.tensor.matmul(sp[:, off:], lhsT=k_sb[h][:, kc:kc + 128],
                                     rhs=q_sb[h][:, qc + off:qc + W],
                                     start=True, stop=True)
                    pb = prp.tile([128, W], BF16, tag="probs", name=f"pb{b}_{h}_{qw}_{c}")
                    nc.scalar.activation(pb[:, off:], sp[:, off:],
                                         mybir.ActivationFunctionType.Exp,
                                         scale=RSQRT_HD)
                    if kind == MASKED and off < W:
                        if tri:
                            hi = min(off + 128, W)
                            nc.vector.tensor_mul(pb[:, off:hi], pb[:, off:hi],
                                                 em_tiles[c][:, :hi - off])
                        else:
                            nc.vector.tensor_mul(pb[:, off:], pb[:, off:],
                                                 em_tiles[c][:, off:])
                    g = b * NKC + c
                    o0 = g * DHC + h * 128
                    nc.tensor.matmul(att[:, off:], lhsT=vT_sb[:, o0:o0 + 128],
                                     rhs=pb[:, off:],
                                     start=(ci == 0), stop=(ci == n - 1))
                    if pe_dsum:
                        nc.tensor.matmul(dsm[:, off:], lhsT=ones, rhs=pb[:, off:],
                                         start=(ci == 0), stop=(ci == n - 1))
                    elif ci == 0:
                        nc.vector.tensor_copy(acc, pb)
                    else:
                        nc.vector.tensor_add(acc[:, off:], acc[:, off:], pb[:, off:])
                if not pe_dsum:
                    nc.tensor.matmul(dsm, lhsT=ones, rhs=acc, start=True, stop=True)
                rc = smp.tile([128, W], F32, tag="recip", name=f"rc{b}_{h}_{qw}")
                with tc.high_priority():
                    nc.vector.reciprocal(rc, dsm)
                    nc.vector.tensor_mul(a_sb[h][:, qc:qc + W], att, rc)

            def outproj_block(st, split_dma=False):
                """Out-projection for one 128-row seq block (both heads)."""
                for dgg in range(2):
                    ops = [psQ.tile([128, W], F32, tag="q", name=f"o{st}_{dgg}_{d2}")
                           for d2 in range(2)]
                    for h in range(HC):
                        for d2 in range(2):
                            dg = dgg * 2 + d2
                            nc.tensor.matmul(
                                ops[d2], lhsT=a_sb[h][:, st * 128:(st + 1) * 128],
                                rhs=woT_sb[h][:, dg * W:(dg + 1) * W],
                                start=(h == 0), stop=(h == HC - 1))
                    for d2 in range(2):
                        dg = dgg * 2 + d2
                        ob = outp.tile([128, W], BF16, tag="ob", name=f"ob{st}_{dg}")
                        with tc.high_priority():
                            nc.scalar.copy(ob, ops[d2])
                        dst = out_d[st * 128:(st + 1) * 128, dg * W:(dg + 1) * W]
                        if split_dma:
                            # split the tail DMAs across queues so the last
                            # store isn't a single serialized transfer
                            nc.sync.dma_start(out=dst[0:64, :], in_=ob[0:64, :])
                            nc.sync.dma_start(out=dst[64:128, :], in_=ob[64:128, :])
                        else:
                            nc.sync.dma_start(out=dst, in_=ob)

            # ---- emission schedule ----
            # QKV for batch 0 back to back; then batch-1 QKV groups interleaved
            # with batch-0 attention windows (fills exp-bound PE bubbles); each
            # window pair is chased by its out-projection blocks.
            for sg in range(2):
                xts = xts_cur
                xts_cur = load_xt_group(sg + 1)
                if sg == 0:
                    nc.scalar.dma_start(out=trig_sb, in_=trig[:, :])
                else:
                    for h in range(HC):
                        nc.scalar.dma_start(out=woT_sb[h],
                                            in_=woT[h * 128:(h + 1) * 128, :])
                for ti in range(3):
                    for dh in range(2):
                        qkv_group(sg, ti, dh, xts)

            # batch-1 QKV (12 groups) interleaved with batch-0 attention
            # (8 windows + em prefetch + outproj chasers)
            b0_units = []
            for qw in range(NQW):
                active, em_tiles = attn_window_em(0, qw)
                for h in range(HC):
                    b0_units.append(("w", 0, qw, h, active, em_tiles))
                if qw >= 1:
                    b0_units.append(("o", 0, qw - 1))
            qkv_units = []
            for sg in range(2, 4):
                qkv_units.append(("x", sg))
                for ti in range(3):
                    for dh in range(2):
                        qkv_units.append(("g", sg, ti, dh))
            qi = wi2 = 0
            sched = []
            while qi < len(qkv_units) or wi2 < len(b0_units):
                if qi < len(qkv_units):
                    sched.append(qkv_units[qi]); qi += 1
                    if qi < len(qkv_units) and qkv_units[qi][0] == "x":
                        sched.append(qkv_units[qi]); qi += 1
                if wi2 < len(b0_units):
                    sched.append(b0_units[wi2]); wi2 += 1
            for u in sched:
                if u[0] == "x":
                    xts = xts_cur
                    if u[1] + 1 < NSG:
                        xts_cur = load_xt_group(u[1] + 1)
                elif u[0] == "g":
                    qkv_group(u[1], u[2], u[3], xts)
                elif u[0] == "w":
                    attn_window(u[1], u[2], u[3], u[4], u[5])
                else:
                    for st in range(u[1] * 16 + u[2] * 4, u[1] * 16 + u[2] * 4 + 4):
                        outproj_block(st)

            # batch-1 attention; outproj chases one window behind so the
            # a_sb normalize latency hides under the next window's work
            for qw in range(NQW):
                active, em_tiles = attn_window_em(1, qw)
                for h in range(HC):
                    attn_window(1, qw, h, active, em_tiles,
                                pe_dsum=(qw == NQW - 1))
                if qw == 0:
                    for st in range(12, 16):          # b0 qw3 blocks
                        outproj_block(st)
                else:
                    for st in range(16 + (qw - 1) * 4, 16 + qw * 4):
                        outproj_block(st)
            for st in range(28, 32):
                outproj_block(st, split_dma=True)
    _split_multi_waits(nc)
    return nc


def _prepare(x, freqs_cos, freqs_sin, mask, wq, wk, wv, wo):
    x = np.asarray(x, dtype=np.float32)
    wq = np.asarray(wq, dtype=np.float32)
    wk = np.asarray(wk, dtype=np.float32)
    wv = np.asarray(wv, dtype=np.float32)
    wo = np.asarray(wo, dtype=np.float32)
    fc = np.asarray(freqs_cos, dtype=np.float32)
    fs = np.asarray(freqs_sin, dtype=np.float32)
    mask = np.asarray(mask, dtype=np.float32)

    xT = np.ascontiguousarray(x.reshape(BS, D).T).astype(NPBF16)

    cosT = fc.T                      # [64, S]
    sinT = fs.T
    cos_dup = np.vstack([cosT, cosT])
    sin_sgn = np.vstack([-sinT, sinT])
    trig = np.ascontiguousarray(np.hstack([cos_dup, sin_sgn])).astype(NPBF16)

    em = np.exp(mask).T              # [k, q]; exp(-inf)=0, exp(0)=1
    emaskT = np.ascontiguousarray(em).astype(NPBF16)
    cls = []
    for qw in range(NQW):
        row = []
        for c in range(NKC):
            t = emaskT[c * 128:(c + 1) * 128, qw * W:(qw + 1) * W]
            if not t.any():
                row.append((SKIP, 0, False))
            elif (t == NPBF16(1.0)).all():
                row.append((FREE, 0, False))
            else:
                colnz = (np.asarray(t, dtype=np.float32) != 0).any(axis=0)
                off = int(np.argmax(colnz))  # first column with any valid entry
                hi = min(off + 128, W)
                tri = bool((t[:, hi:] == NPBF16(1.0)).all())
                row.append((MASKED, off, tri))
        cls.append(tuple(row))
    cls_key = tuple(cls)

    def chunk_major(wT):
        # [D, DHC] -> [128, NDIN*DHC] where [p, di*DHC+c] = wT[di*128+p, c]
        return np.ascontiguousarray(
            wT.reshape(NDIN, 128, DHC).transpose(1, 0, 2).reshape(128, NDIN * DHC)
        ).astype(NPBF16)

    # deinterleave perm: even dims then odd dims, per head
    ridx = np.concatenate([np.arange(0, HD, 2), np.arange(1, HD, 2)])
    in_maps = []
    for core in range(NCORES):
        heads = [core * HC + h for h in range(HC)]
        qk_rows = np.concatenate([g * HD + ridx for g in heads])
        v_rows = np.concatenate([np.arange(g * HD, (g + 1) * HD) for g in heads])
        m = {
            "xT": xT,
            "wqT": chunk_major(wq[qk_rows].T),
            "wkT": chunk_major(wk[qk_rows].T),
            "wvT": chunk_major(wv[v_rows].T),
            "woT": np.ascontiguousarray(wo[:, v_rows].T).astype(NPBF16),
            "trig": trig,
            "emaskT": emaskT,
        }
        in_maps.append(m)
    return in_maps, cls_key


def kernel(x, start_pos, freqs_cos, freqs_sin, mask, wq, wk, wv, wo):
    in_maps, cls_key = _prepare(x, freqs_cos, freqs_sin, mask, wq, wk, wv, wo)
    nc = _PROGRAM_CACHE.get(cls_key)
    if nc is None:
        nc = _build(cls_key)
        _PROGRAM_CACHE[cls_key] = nc
    res = run_bass_kernel_spmd(
        nc, in_maps, list(range(NCORES)),
        trace=bool(os.environ.get("KERNEL_TRACE")),
        tmpdir=os.environ.get("KERNEL_TRACE_DIR") or None)
    LAST_RUN[0] = res
    out = np.zeros([BS, D], np.float32)
    for r in res.results:
        out += np.asarray(r["out"], dtype=np.float32)
    return out.reshape(B, S, D)


# revision 11
# speedup vs baseline: 1.0170x; 1.0170x over previous
"""Tensor-parallel attention block (QKV + RoPE + causal attention + out-proj)
for 8 Trainium2 NeuronCores.

Sharding: heads (16) split across 8 cores, 2 heads/core. wq/wk/wv column-
sharded, wo row-sharded; each core computes a full-shape partial output and
the host sums the 8 partials.

Layout trick: everything on the PE array is a natural `lhsT.T @ rhs`:
  - host pre-transposes x -> xT [D, B*S] so projections need no transposes
  - q,k produced in [head_dim, seq] layout; scores computed TRANSPOSED
    ([k_seq, q_seq]) so softmax needs no on-chip transposes at all
  - softmax denominator: DVE accumulates exp'd prob chunks into an f32 SBUF
    tile, then ONE ones-matmul per q-window gives the partition sum (keeps
    the PE out of the per-chunk denominator business)
  - RoPE pair-halves are deinterleaved by permuting wq/wk rows on host;
    the half-swap needed by rotation is a chunked SBUF->SBUF DMA pipelined
    right behind each seq-group's projections
  - mask handled exactly as multiplicative exp(mask) tiles; all-zero tiles
    skip compute, all-one tiles skip the multiply (derived from the real
    mask values at build time, not assumed causal)
"""

import math
import os
import sys

import numpy as np
import ml_dtypes

sys.path.insert(0, "/opt/trn_rl_repo")

import concourse.bass as bass
import concourse.mybir as mybir
from concourse.tile import TileContext
from concourse.bass_utils import run_bass_kernel_spmd
from concourse.masks import make_identity

B, S, D, H = 2, 2048, 2048, 16
HD = D // H            # 128 head dim
NCORES = 8
HC = H // NCORES       # 2 heads per core
DHC = HC * HD          # 256
BS = B * S             # 4096
NDIN = D // 128        # 16 contraction chunks
W = 512                # attention q-window / matmul free size
NQW = S // W           # 4 q windows per batch
NKC = S // 128         # 16 k chunks per batch
SG = 1024              # qkv seq-group width
NSG = BS // SG         # 4
RSQRT_HD = 1.0 / math.sqrt(HD)
NWARM = 150            # PE warmup matmuls (keep HAM at 8/8 until data lands)

BF16 = mybir.dt.bfloat16
F32 = mybir.dt.float32
NPBF16 = ml_dtypes.bfloat16

SKIP, FREE, MASKED = 0, 1, 2

# stash of the last BassKernelResults for the test harness (exec_time_ns etc)
LAST_RUN = [None]
_PROGRAM_CACHE = {}


def _split_multi_waits(nc):
    """Walrus codegen allows only 1 embedded sync-wait per instruction (2 for
    EventSemaphore). Tile's sem-assignment can emit more; hoist the excess into
    standalone InstEventSemaphore waits on the same engine, just before."""
    n = 0
    for blk in nc.m.functions[0].blocks:
        out = []
        for inst in blk.instructions:
            si = getattr(inst, "sync_info", None)
            cap = 2 if isinstance(inst, mybir.InstEventSemaphore) else 1
            if si is not None and si.on_wait and len(si.on_wait) > cap:
                waits = list(si.on_wait)
                for w in waits[:-cap]:
                    n += 1
                    ev = mybir.InstEventSemaphore(
                        name=f"{inst.name}_xw{n}",
                        ins=[], outs=[],
                        sync_info=mybir.SyncInfo(on_wait=[w], on_update=[]))
                    ev.engine = inst.engine
                    out.append(ev)
                si.on_wait = waits[-cap:]
            out.append(inst)
        blk.instructions = out


def _build(cls_key):
    """Build the per-core Bass program. cls_key: tuple[NQW][NKC] of SKIP/FREE/MASKED."""
    cls = [list(row) for row in cls_key]
    nc = bass.Bass()

    xT = nc.declare_dram_parameter("xT", [D, BS], BF16, isOutput=False)
    # weights chunk-major: [128, NDIN, DHC] flattened, quad q holds di 4q..4q+3
    wqT = nc.declare_dram_parameter("wqT", [128, NDIN * DHC], BF16, isOutput=False)
    wkT = nc.declare_dram_parameter("wkT", [128, NDIN * DHC], BF16, isOutput=False)
    wvT = nc.declare_dram_parameter("wvT", [128, NDIN * DHC], BF16, isOutput=False)
    woT = nc.declare_dram_parameter("woT", [DHC, D], BF16, isOutput=False)
    trig = nc.declare_dram_parameter("trig", [128, 2 * S], BF16, isOutput=False)
    emaskT = nc.declare_dram_parameter("emaskT", [S, S], BF16, isOutput=False)
    out_d = nc.declare_dram_parameter("out", [BS, D], BF16, isOutput=True)

    with TileContext(nc) as tc:
        with (
            tc.tile_pool(name="consts", bufs=1) as consts,
            tc.tile_pool(name="xt", bufs=5) as xtp,
            tc.tile_pool(name="rsw", bufs=4) as rswp,
            tc.tile_pool(name="rm", bufs=4) as rmp,
            tc.tile_pool(name="vtmp", bufs=2) as vtp,
            tc.tile_pool(name="probs", bufs=5) as prp,
            tc.tile_pool(name="acc", bufs=3) as accp,
            tc.tile_pool(name="emask", bufs=6) as emp,
            tc.tile_pool(name="small", bufs=2) as smp,
            tc.tile_pool(name="outsb", bufs=3) as outp,
            tc.tile_pool(name="psQ", bufs=3, space="PSUM") as psQ,
            tc.tile_pool(name="psS", bufs=2, space="PSUM") as psS,
            tc.tile_pool(name="psC", bufs=2, space="PSUM") as psC,
        ):
            # persistent tiles
            q_sb = [consts.tile([128, BS], BF16, tag=f"q{h}", name=f"q{h}") for h in range(HC)]
            k_sb = [consts.tile([128, BS], BF16, tag=f"k{h}", name=f"k{h}") for h in range(HC)]
            a_sb = [consts.tile([128, BS], BF16, tag=f"a{h}", name=f"a{h}") for h in range(HC)]
            vT_sb = consts.tile([128, B * NKC * DHC], BF16, tag="vT", name="vT")
            ident = consts.tile([128, 128], BF16, tag="ident", name="ident")
            make_identity(nc, ident)
            ones = consts.tile([128, 128], BF16, tag="ones", name="ones")
            nc.vector.memset(ones, 1.0)

            # qkv weights: 4-chunk quads so the first matmul only waits on
            # 256 KB, not the full megabyte. wt[ti][q][:, j, :] = chunk 4q+j.
            wt = []
            for wi_, wparam in enumerate([wqT, wkT, wvT]):
                wv_ = wparam.rearrange("p (n m) -> p n m", n=NDIN)
                quads = []
                for qd in range(4):
                    t_ = consts.tile([128, 4, DHC], BF16, tag=f"w{wi_}_{qd}",
                                     name=f"w{wi_}_{qd}")
                    quads.append(t_)
                wt.append(quads)
            # DMA order: everything QKV needs for sg0 first, in consumption order
            nc.sync.dma_start(out=wt[0][0], in_=wqT.rearrange(
                "p (n m) -> p n m", n=NDIN)[:, 0:4, :])
            # warm the PE clock (HAM releases the 1.2GHz throttle after ~3.4us
            # of sustained activity) while the first DMAs are in flight
            wu = psC.tile([128, 128], F32, tag="dsum", bufs=1, name="warmup")
            for i in range(NWARM):
                nc.tensor.matmul(wu, lhsT=ones, rhs=ones, start=True, stop=True)

            def load_xt_group(sg, fine=False):
                xts = []
                for dj in range(4):
                    tb = xtp.tile([128, 4, SG], BF16, tag="xt", name=f"xt{sg}_{dj}")
                    src = xT[dj * 512:(dj + 1) * 512,
                             sg * SG:(sg + 1) * SG].rearrange("(n p) m -> p n m", p=128)
                    for k4 in range(4):
                        eng = nc.sync if (dj * 4 + k4) % 2 == 0 else nc.scalar
                        if fine and dj < 2:
                            eng.dma_start(out=tb[:, k4, 0:W], in_=src[:, k4, 0:W])
                            eng.dma_start(out=tb[:, k4, W:SG], in_=src[:, k4, W:SG])
                        else:
                            eng.dma_start(out=tb[:, k4, :], in_=src[:, k4, :])
                        xts.append(tb[:, k4, :])
                return xts

            xts_cur = load_xt_group(0, fine=True)
            for wi_ in range(3):
                for qd in range(4):
                    if wi_ == 0 and qd == 0:
                        continue
                    nc.gpsimd.dma_start(
                        out=wt[wi_][qd],
                        in_=[wqT, wkT, wvT][wi_].rearrange(
                            "p (n m) -> p n m", n=NDIN)[:, 4 * qd:4 * qd + 4, :])
            trig_sb = consts.tile([128, 2 * S], BF16, tag="trig", name="trig")
            woT_sb = [consts.tile([128, D], BF16, tag=f"wo{h}", name=f"wo{h}")
                      for h in range(HC)]

            def rope_chunk(tens, h, b, cc, width):
                """Rotate tens[h][:, cc:cc+width] in place (cc global col)."""
                src = tens[h]
                sp = cc - b * S  # position within the batch for trig lookup
                sw = rswp.tile([128, W], BF16, tag="rsw", name=f"sw{cc}_{h}")
                nc.gpsimd.dma_start(out=sw[0:64, :width], in_=src[64:128, cc:cc + width])
                nc.gpsimd.dma_start(out=sw[64:128, :width], in_=src[0:64, cc:cc + width])
                mcc = rmp.tile([128, W], BF16, tag="mcc", name=f"mcc{cc}_{h}")
                mss = rmp.tile([128, W], BF16, tag="mss", name=f"mss{cc}_{h}")
                nc.vector.tensor_mul(mcc[:, :width], src[:, cc:cc + width],
                                     trig_sb[:, sp:sp + width])
                nc.vector.tensor_mul(mss[:, :width], sw[:, :width],
                                     trig_sb[:, S + sp:S + sp + width])
                nc.vector.tensor_add(src[:, cc:cc + width], mcc[:, :width],
                                     mss[:, :width])

            def qkv_group(sg, ti, dh, xts):
                """One projection group: 32 matmuls -> 2 psum tiles -> copies."""
                ps = [psQ.tile([128, W], F32, tag="q", name=f"psA{sg}_{ti}_{dh}_{wi}")
                      for wi in range(2)]
                for di in range(NDIN):
                    for wi in range(2):
                        nc.tensor.matmul(
                            ps[wi], lhsT=wt[ti][di // 4][:, di % 4, dh * 128:(dh + 1) * 128],
                            rhs=xts[di][:, wi * W:(wi + 1) * W],
                            start=(di == 0), stop=(di == NDIN - 1))
                for wi in range(2):
                    c0 = sg * SG + wi * W
                    if ti < 2:
                        dst = (q_sb if ti == 0 else k_sb)[dh]
                        with tc.high_priority():
                            if wi == 0:
                                nc.scalar.copy(dst[:, c0:c0 + W], ps[wi])
                            else:
                                nc.vector.tensor_copy(dst[:, c0:c0 + W], ps[wi])
                    else:
                        vt = vtp.tile([128, W], BF16, tag="vtmp", name=f"vt{sg}_{dh}_{wi}")
                        with tc.high_priority():
                            nc.scalar.copy(vt, ps[wi])
                        for j in range(W // 128):
                            pt = psC.tile([128, 128], BF16, tag="att",
                                          name=f"pvt{sg}_{dh}_{wi}_{j}")
                            nc.tensor.transpose(pt, vt[:, j * 128:(j + 1) * 128], ident)
                            g = (c0 + j * 128) // 128
                            o0 = g * DHC + dh * 128
                            nc.scalar.copy(vT_sb[:, o0:o0 + 128], pt)
                # pipeline RoPE right behind each projection so batch-1
                # scores aren't gated on a late half-swap DMA
                if ti < 2 and dh == 1:
                    tens = q_sb if ti == 0 else k_sb
                    b = sg // 2
                    for dh_ in range(2):
                        for wi in range(2):
                            rope_chunk(tens, dh_, b, sg * SG + wi * W, W)

            def attn_window_em(b, qw):
                active = [c for c in range(NKC) if cls[qw][c][0] != SKIP]
                em_tiles = {}
                for c in active:
                    if cls[qw][c][0] == MASKED:
                        kind, off, tri = cls[qw][c]
                        if tri:
                            # only the 128-wide diagonal band is partial
                            hi = min(off + 128, W)
                            em = emp.tile([128, 128], BF16, tag="em", name=f"em{b}_{qw}_{c}")
                            nc.gpsimd.dma_start(
                                out=em[:, :hi - off],
                                in_=emaskT[c * 128:(c + 1) * 128,
                                           qw * W + off:qw * W + hi])
                        else:
                            em = emp.tile([128, W], BF16, tag="em", name=f"em{b}_{qw}_{c}")
                            nc.gpsimd.dma_start(
                                out=em,
                                in_=emaskT[c * 128:(c + 1) * 128, qw * W:(qw + 1) * W])
                        em_tiles[c] = em
                return active, em_tiles

            def attn_window(b, qw, h, active, em_tiles, pe_dsum=False):
                """Scores -> exp -> AV accumulate -> denominator -> normalize
                for one 512-wide q window of one head. Masked (diagonal) tiles
                only compute columns >= off: columns below the first unmasked
                one are exactly zero and contribute nothing downstream."""
                qc = b * S + qw * W
                att = psC.tile([128, W], F32, tag="att", name=f"att{b}_{h}_{qw}")
                dsm = psC.tile([128, W], F32, tag="dsum", bufs=1, name=f"dsm{b}_{h}_{qw}")
                acc = None
                if not pe_dsum:
                    acc = accp.tile([128, W], BF16, tag="acc", name=f"acc{b}_{h}_{qw}")
                n = len(active)
                for ci, c in enumerate(active):
                    kind, off, tri = cls[qw][c]
                    if kind != MASKED:
                        off = 0
                    wd = W - off
                    sp = psS.tile([128, W], F32, tag="s", name=f"sc{b}_{h}_{qw}_{c}")
                    kc = b * S + c * 128
                    nc.tensor.matmul(sp[:, off:], lhsT=k_sb[h][:, kc:kc + 128],
                                     rhs=q_sb[h][:, qc + off:qc + W],
                                     start=True, stop=True)
                    pb = prp.tile([128, W], BF16, tag="probs", name=f"pb{b}_{h}_{qw}_{c}")
                    nc.scalar.activation(pb[:, off:], sp[:, off:],
                                         mybir.ActivationFunctionType.Exp,
                                         scale=RSQRT_HD)
                    if kind == MASKED and off < W:
                        if tri:
                            hi = min(off + 128, W)
                            nc.vector.tensor_mul(pb[:, off:hi], pb[:, off:hi],
                                                 em_tiles[c][:, :hi - off])
                        else:
                            nc.vector.tensor_mul(pb[:, off:], pb[:, off:],
                                                 em_tiles[c][:, off:])
                    g = b * NKC + c
                    o0 = g * DHC + h * 128
                    nc.tensor.matmul(att[:, off:], lhsT=vT_sb[:, o0:o0 + 128],
                                     rhs=pb[:, off:],
                                     start=(ci == 0), stop=(ci == n - 1))
                    if pe_dsum:
                        nc.tensor.matmul(dsm[:, off:], lhsT=ones, rhs=pb[:, off:],
                                         start=(ci == 0), stop=(ci == n - 1))
                    elif ci == 0:
                        nc.vector.tensor_copy(acc, pb)
                    else:
                        nc.vector.tensor_add(acc[:, off:], acc[:, off:], pb[:, off:])
                if not pe_dsum:
                    nc.tensor.matmul(dsm, lhsT=ones, rhs=acc, start=True, stop=True)
                rc = smp.tile([128, W], F32, tag="recip", name=f"rc{b}_{h}_{qw}")
                with tc.high_priority():
                    nc.vector.reciprocal(rc, dsm)
                    nc.vector.tensor_mul(a_sb[h][:, qc:qc + W], att, rc)

            def outproj_block(st, split_dma=False):
                """Out-projection for one 128-row seq block (both heads)."""
                for dgg in range(2):
                    ops = [psQ.tile([128, W], F32, tag="q", name=f"o{st}_{dgg}_{d2}")
                           for d2 in range(2)]
                    for h in range(HC):
                        for d2 in range(2):
                            dg = dgg * 2 + d2
                            nc.tensor.matmul(
                                ops[d2], lhsT=a_sb[h][:, st * 128:(st + 1) * 128],
                                rhs=woT_sb[h][:, dg * W:(dg + 1) * W],
                                start=(h == 0), stop=(h == HC - 1))
                    for d2 in range(2):
                        dg = dgg * 2 + d2
                        ob = outp.tile([128, W], BF16, tag="ob", name=f"ob{st}_{dg}")
                        with tc.high_priority():
                            if (st + dg) % 2 == 0:
                                nc.scalar.copy(ob, ops[d2])
                            else:
                                nc.vector.tensor_copy(ob, ops[d2])
                        dst = out_d[st * 128:(st + 1) * 128, dg * W:(dg + 1) * W]
                        if split_dma:
                            # split the tail DMAs across queues so the last
                            # store isn't a single serialized transfer
                            nc.sync.dma_start(out=dst[0:64, :], in_=ob[0:64, :])
                            nc.sync.dma_start(out=dst[64:128, :], in_=ob[64:128, :])
                        else:
                            nc.sync.dma_start(out=dst, in_=ob)

            # ---- emission schedule ----
            # QKV for batch 0 back to back; then batch-1 QKV groups interleaved
            # with batch-0 attention windows (fills exp-bound PE bubbles); each
            # window pair is chased by its out-projection blocks.
            for sg in range(2):
                xts = xts_cur
                xts_cur = load_xt_group(sg + 1)
                if sg == 0:
                    nc.scalar.dma_start(out=trig_sb, in_=trig[:, :])
                else:
                    for h in range(HC):
                        nc.scalar.dma_start(out=woT_sb[h],
                                            in_=woT[h * 128:(h + 1) * 128, :])
                for ti in range(3):
                    for dh in range(2):
                        qkv_group(sg, ti, dh, xts)

            # batch-1 QKV (12 groups) interleaved with batch-0 attention
            # (8 windows + em prefetch + outproj chasers)
            b0_units = []
            for qw in range(NQW):
                active, em_tiles = attn_window_em(0, qw)
                for h in range(HC):
                    b0_units.append(("w", 0, qw, h, active, em_tiles))
                if qw >= 1:
                    b0_units.append(("o", 0, qw - 1))
            qkv_units = []
            for sg in range(2, 4):
                qkv_units.append(("x", sg))
                for ti in range(3):
                    for dh in range(2):
                        qkv_units.append(("g", sg, ti, dh))
            qi = wi2 = 0
            sched = []
            while qi < len(qkv_units) or wi2 < len(b0_units):
                if qi < len(qkv_units):
                    sched.append(qkv_units[qi]); qi += 1
                    if qi < len(qkv_units) and qkv_units[qi][0] == "x":
                        sched.append(qkv_units[qi]); qi += 1
                if wi2 < len(b0_units):
                    sched.append(b0_units[wi2]); wi2 += 1
            for u in sched:
                if u[0] == "x":
                    xts = xts_cur
                    if u[1] + 1 < NSG:
                        xts_cur = load_xt_group(u[1] + 1)
                elif u[0] == "g":
                    qkv_group(u[1], u[2], u[3], xts)
                elif u[0] == "w":
                    attn_window(u[1], u[2], u[3], u[4], u[5])
                else:
                    for st in range(u[1] * 16 + u[2] * 4, u[1] * 16 + u[2] * 4 + 4):
                        outproj_block(st)

            # batch-1 attention; outproj chases one window behind so the
            # a_sb normalize latency hides under the next window's work
            for qw in range(NQW):
                active, em_tiles = attn_window_em(1, qw)
                for h in range(HC):
                    attn_window(1, qw, h, active, em_tiles,
                                pe_dsum=(qw == NQW - 1))
                if qw == 0:
                    for st in range(12, 16):          # b0 qw3 blocks
                        outproj_block(st)
                else:
                    for st in range(16 + (qw - 1) * 4, 16 + qw * 4):
                        outproj_block(st)
            for st in range(28, 32):
                outproj_block(st, split_dma=True)
    _split_multi_waits(nc)
    return nc


def _prepare(x, freqs_cos, freqs_sin, mask, wq, wk, wv, wo):
    x = np.asarray(x, dtype=np.float32)
    wq = np.asarray(wq, dtype=np.float32)
    wk = np.asarray(wk, dtype=np.float32)
    wv = np.asarray(wv, dtype=np.float32)
    wo = np.asarray(wo, dtype=np.float32)
    fc = np.asarray(freqs_cos, dtype=np.float32)
    fs = np.asarray(freqs_sin, dtype=np.float32)
    mask = np.asarray(mask, dtype=np.float32)

    xT = np.ascontiguousarray(x.reshape(BS, D).T).astype(NPBF16)

    cosT = fc.T                      # [64, S]
    sinT = fs.T
    cos_dup = np.vstack([cosT, cosT])
    sin_sgn = np.vstack([-sinT, sinT])
    trig = np.ascontiguousarray(np.hstack([cos_dup, sin_sgn])).astype(NPBF16)

    em = np.exp(mask).T              # [k, q]; exp(-inf)=0, exp(0)=1
    emaskT = np.ascontiguousarray(em).astype(NPBF16)
    cls = []
    for qw in range(NQW):
        row = []
        for c in range(NKC):
            t = emaskT[c * 128:(c + 1) * 128, qw * W:(qw + 1) * W]
            if not t.any():
                row.append((SKIP, 0, False))
            elif (t == NPBF16(1.0)).all():
                row.append((FREE, 0, False))
            else:
                colnz = (np.asarray(t, dtype=np.float32) != 0).any(axis=0)
                off = int(np.argmax(colnz))  # first column with any valid entry
                hi = min(off + 128, W)
                tri = bool((t[:, hi:] == NPBF16(1.0)).all())
                row.append((MASKED, off, tri))
        cls.append(tuple(row))
    cls_key = tuple(cls)

    def chunk_major(wT):
        # [D, DHC] -> [128, NDIN*DHC] where [p, di*DHC+c] = wT[di*128+p, c]
        return np.ascontiguousarray(
            wT.reshape(NDIN, 128, DHC).transpose(1, 0, 2).reshape(128, NDIN * DHC)
        ).astype(NPBF16)

    # deinterleave perm: even dims then odd dims, per head
    ridx = np.concatenate([np.arange(0, HD, 2), np.arange(1, HD, 2)])
    in_maps = []
    for core in range(NCORES):
        heads = [core * HC + h for h in range(HC)]
        qk_rows = np.concatenate([g * HD + ridx for g in heads])
        v_rows = np.concatenate([np.arange(g * HD, (g + 1) * HD) for g in heads])
        m = {
            "xT": xT,
            "wqT": chunk_major(wq[qk_rows].T),
            "wkT": chunk_major(wk[qk_rows].T),
            "wvT": chunk_major(wv[v_rows].T),
            "woT": np.ascontiguousarray(wo[:, v_rows].T).astype(NPBF16),
            "trig": trig,
            "emaskT": emaskT,
        }
        in_maps.append(m)
    return in_maps, cls_key


def kernel(x, start_pos, freqs_cos, freqs_sin, mask, wq, wk, wv, wo):
    in_maps, cls_key = _prepare(x, freqs_cos, freqs_sin, mask, wq, wk, wv, wo)
    nc = _PROGRAM_CACHE.get(cls_key)
    if nc is None:
        nc = _build(cls_key)
        _PROGRAM_CACHE[cls_key] = nc
    res = run_bass_kernel_spmd(
        nc, in_maps, list(range(NCORES)),
        trace=bool(os.environ.get("KERNEL_TRACE")),
        tmpdir=os.environ.get("KERNEL_TRACE_DIR") or None)
    LAST_RUN[0] = res
    out = np.zeros([BS, D], np.float32)
    for r in res.results:
        out += np.asarray(r["out"], dtype=np.float32)
    return out.reshape(B, S, D)


# revision 12
# speedup vs baseline: 1.0305x; 1.0132x over previous
"""Tensor-parallel attention block (QKV + RoPE + causal attention + out-proj)
for 8 Trainium2 NeuronCores.

Sharding: heads (16) split across 8 cores, 2 heads/core. wq/wk/wv column-
sharded, wo row-sharded; each core computes a full-shape partial output and
the host sums the 8 partials.

Layout trick: everything on the PE array is a natural `lhsT.T @ rhs`:
  - host pre-transposes x -> xT [D, B*S] so projections need no transposes
  - q,k produced in [head_dim, seq] layout; scores computed TRANSPOSED
    ([k_seq, q_seq]) so softmax needs no on-chip transposes at all
  - softmax denominator: DVE accumulates exp'd prob chunks into an f32 SBUF
    tile, then ONE ones-matmul per q-window gives the partition sum (keeps
    the PE out of the per-chunk denominator business)
  - RoPE pair-halves are deinterleaved by permuting wq/wk rows on host;
    the half-swap needed by rotation is a chunked SBUF->SBUF DMA pipelined
    right behind each seq-group's projections
  - mask handled exactly as multiplicative exp(mask) tiles; all-zero tiles
    skip compute, all-one tiles skip the multiply (derived from the real
    mask values at build time, not assumed causal)
"""

import math
import os
import sys

import numpy as np
import ml_dtypes

sys.path.insert(0, "/opt/trn_rl_repo")

import concourse.bass as bass
import concourse.mybir as mybir
from concourse.tile import TileContext
from concourse.bass_utils import run_bass_kernel_spmd
from concourse.masks import make_identity

B, S, D, H = 2, 2048, 2048, 16
HD = D // H            # 128 head dim
NCORES = 8
HC = H // NCORES       # 2 heads per core
DHC = HC * HD          # 256
BS = B * S             # 4096
NDIN = D // 128        # 16 contraction chunks
W = 512                # attention q-window / matmul free size
NQW = S // W           # 4 q windows per batch
NKC = S // 128         # 16 k chunks per batch
SG = 1024              # qkv seq-group width
NSG = BS // SG         # 4
RSQRT_HD = 1.0 / math.sqrt(HD)
NWARM = 150            # PE warmup matmuls (keep HAM at 8/8 until data lands)

BF16 = mybir.dt.bfloat16
F32 = mybir.dt.float32
NPBF16 = ml_dtypes.bfloat16

SKIP, FREE, MASKED = 0, 1, 2

# stash of the last BassKernelResults for the test harness (exec_time_ns etc)
LAST_RUN = [None]
_PROGRAM_CACHE = {}


def _split_multi_waits(nc):
    """Walrus codegen allows only 1 embedded sync-wait per instruction (2 for
    EventSemaphore). Tile's sem-assignment can emit more; hoist the excess into
    standalone InstEventSemaphore waits on the same engine, just before."""
    n = 0
    for blk in nc.m.functions[0].blocks:
        out = []
        for inst in blk.instructions:
            si = getattr(inst, "sync_info", None)
            cap = 2 if isinstance(inst, mybir.InstEventSemaphore) else 1
            if si is not None and si.on_wait and len(si.on_wait) > cap:
                waits = list(si.on_wait)
                for w in waits[:-cap]:
                    n += 1
                    ev = mybir.InstEventSemaphore(
                        name=f"{inst.name}_xw{n}",
                        ins=[], outs=[],
                        sync_info=mybir.SyncInfo(on_wait=[w], on_update=[]))
                    ev.engine = inst.engine
                    out.append(ev)
                si.on_wait = waits[-cap:]
            out.append(inst)
        blk.instructions = out


def _build(cls_key):
    """Build the per-core Bass program. cls_key: tuple[NQW][NKC] of SKIP/FREE/MASKED."""
    cls = [list(row) for row in cls_key]
    nc = bass.Bass()

    xT = nc.declare_dram_parameter("xT", [D, BS], BF16, isOutput=False)
    # weights chunk-major: [128, NDIN, DHC] flattened, quad q holds di 4q..4q+3
    wqT = nc.declare_dram_parameter("wqT", [128, NDIN * DHC], BF16, isOutput=False)
    wkT = nc.declare_dram_parameter("wkT", [128, NDIN * DHC], BF16, isOutput=False)
    wvT = nc.declare_dram_parameter("wvT", [128, NDIN * DHC], BF16, isOutput=False)
    woT = nc.declare_dram_parameter("woT", [DHC, D], BF16, isOutput=False)
    trig = nc.declare_dram_parameter("trig", [128, 2 * S], BF16, isOutput=False)
    emaskT = nc.declare_dram_parameter("emaskT", [S, S], BF16, isOutput=False)
    out_d = nc.declare_dram_parameter("out", [BS, D], BF16, isOutput=True)

    with TileContext(nc) as tc:
        with (
            tc.tile_pool(name="consts", bufs=1) as consts,
            tc.tile_pool(name="xt", bufs=5) as xtp,
            tc.tile_pool(name="rsw", bufs=4) as rswp,
            tc.tile_pool(name="rm", bufs=4) as rmp,
            tc.tile_pool(name="vtmp", bufs=2) as vtp,
            tc.tile_pool(name="probs", bufs=5) as prp,
            tc.tile_pool(name="acc", bufs=3) as accp,
            tc.tile_pool(name="emask", bufs=6) as emp,
            tc.tile_pool(name="small", bufs=2) as smp,
            tc.tile_pool(name="outsb", bufs=3) as outp,
            tc.tile_pool(name="psQ", bufs=3, space="PSUM") as psQ,
            tc.tile_pool(name="psS", bufs=2, space="PSUM") as psS,
            tc.tile_pool(name="psC", bufs=2, space="PSUM") as psC,
        ):
            # persistent tiles
            q_sb = [consts.tile([128, BS], BF16, tag=f"q{h}", name=f"q{h}") for h in range(HC)]
            k_sb = [consts.tile([128, BS], BF16, tag=f"k{h}", name=f"k{h}") for h in range(HC)]
            a_sb = [consts.tile([128, BS], BF16, tag=f"a{h}", name=f"a{h}") for h in range(HC)]
            vT_sb = consts.tile([128, B * NKC * DHC], BF16, tag="vT", name="vT")
            ident = consts.tile([128, 128], BF16, tag="ident", name="ident")
            make_identity(nc, ident)
            ones = consts.tile([128, 128], BF16, tag="ones", name="ones")
            nc.vector.memset(ones, 1.0)

            # qkv weights: 4-chunk quads so the first matmul only waits on
            # 256 KB, not the full megabyte. wt[ti][q][:, j, :] = chunk 4q+j.
            wt = []
            for wi_, wparam in enumerate([wqT, wkT, wvT]):
                wv_ = wparam.rearrange("p (n m) -> p n m", n=NDIN)
                quads = []
                for qd in range(4):
                    t_ = consts.tile([128, 4, DHC], BF16, tag=f"w{wi_}_{qd}",
                                     name=f"w{wi_}_{qd}")
                    quads.append(t_)
                wt.append(quads)
            # DMA order: everything QKV needs for sg0 first, in consumption order
            nc.sync.dma_start(out=wt[0][0], in_=wqT.rearrange(
                "p (n m) -> p n m", n=NDIN)[:, 0:4, :])
            # warm the PE clock (HAM releases the 1.2GHz throttle after ~3.4us
            # of sustained activity) while the first DMAs are in flight
            wu = psC.tile([128, 128], F32, tag="dsum", bufs=1, name="warmup")
            for i in range(NWARM):
                nc.tensor.matmul(wu, lhsT=ones, rhs=ones, start=True, stop=True)

            def load_xt_group(sg, fine=False):
                xts = []
                for dj in range(4):
                    tb = xtp.tile([128, 4, SG], BF16, tag="xt", name=f"xt{sg}_{dj}")
                    src = xT[dj * 512:(dj + 1) * 512,
                             sg * SG:(sg + 1) * SG].rearrange("(n p) m -> p n m", p=128)
                    for k4 in range(4):
                        eng = nc.sync if (dj * 4 + k4) % 2 == 0 else nc.scalar
                        if fine and dj < 2:
                            eng.dma_start(out=tb[:, k4, 0:W], in_=src[:, k4, 0:W])
                            eng.dma_start(out=tb[:, k4, W:SG], in_=src[:, k4, W:SG])
                        else:
                            eng.dma_start(out=tb[:, k4, :], in_=src[:, k4, :])
                        xts.append(tb[:, k4, :])
                return xts

            xts_cur = load_xt_group(0, fine=True)
            for wi_ in range(3):
                for qd in range(4):
                    if wi_ == 0 and qd == 0:
                        continue
                    nc.gpsimd.dma_start(
                        out=wt[wi_][qd],
                        in_=[wqT, wkT, wvT][wi_].rearrange(
                            "p (n m) -> p n m", n=NDIN)[:, 4 * qd:4 * qd + 4, :])
            trig_sb = consts.tile([128, 2 * S], BF16, tag="trig", name="trig")
            woT_sb = [consts.tile([128, D], BF16, tag=f"wo{h}", name=f"wo{h}")
                      for h in range(HC)]

            def rope_chunk(tens, h, b, cc, width):
                """Rotate tens[h][:, cc:cc+width] in place (cc global col)."""
                src = tens[h]
                sp = cc - b * S  # position within the batch for trig lookup
                sw = rswp.tile([128, W], BF16, tag="rsw", name=f"sw{cc}_{h}")
                nc.gpsimd.dma_start(out=sw[0:64, :width], in_=src[64:128, cc:cc + width])
                nc.gpsimd.dma_start(out=sw[64:128, :width], in_=src[0:64, cc:cc + width])
                mcc = rmp.tile([128, W], BF16, tag="mcc", name=f"mcc{cc}_{h}")
                mss = rmp.tile([128, W], BF16, tag="mss", name=f"mss{cc}_{h}")
                nc.vector.tensor_mul(mcc[:, :width], src[:, cc:cc + width],
                                     trig_sb[:, sp:sp + width])
                nc.vector.tensor_mul(mss[:, :width], sw[:, :width],
                                     trig_sb[:, S + sp:S + sp + width])
                nc.vector.tensor_add(src[:, cc:cc + width], mcc[:, :width],
                                     mss[:, :width])

            def qkv_group(sg, ti, dh, xts):
                """One projection group: 32 matmuls -> 2 psum tiles -> copies."""
                ps = [psQ.tile([128, W], F32, tag="q", name=f"psA{sg}_{ti}_{dh}_{wi}")
                      for wi in range(2)]
                for di in range(NDIN):
                    for wi in range(2):
                        nc.tensor.matmul(
                            ps[wi], lhsT=wt[ti][di // 4][:, di % 4, dh * 128:(dh + 1) * 128],
                            rhs=xts[di][:, wi * W:(wi + 1) * W],
                            start=(di == 0), stop=(di == NDIN - 1))
                for wi in range(2):
                    c0 = sg * SG + wi * W
                    if ti < 2:
                        dst = (q_sb if ti == 0 else k_sb)[dh]
                        with tc.high_priority():
                            nc.scalar.copy(dst[:, c0:c0 + W], ps[wi])
                    else:
                        vt = vtp.tile([128, W], BF16, tag="vtmp", name=f"vt{sg}_{dh}_{wi}")
                        with tc.high_priority():
                            nc.scalar.copy(vt, ps[wi])
                        for j in range(W // 128):
                            pt = psC.tile([128, 128], BF16, tag="att",
                                          name=f"pvt{sg}_{dh}_{wi}_{j}")
                            nc.tensor.transpose(pt, vt[:, j * 128:(j + 1) * 128], ident)
                            g = (c0 + j * 128) // 128
                            o0 = g * DHC + dh * 128
                            nc.scalar.copy(vT_sb[:, o0:o0 + 128], pt)
                # pipeline RoPE right behind each projection so batch-1
                # scores aren't gated on a late half-swap DMA
                if ti < 2 and dh == 1:
                    tens = q_sb if ti == 0 else k_sb
                    b = sg // 2
                    for dh_ in range(2):
                        for wi in range(2):
                            rope_chunk(tens, dh_, b, sg * SG + wi * W, W)

            def attn_window_em(b, qw):
                active = [c for c in range(NKC) if cls[qw][c][0] != SKIP]
                em_tiles = {}
                for c in active:
                    if cls[qw][c][0] == MASKED:
                        kind, off, tri = cls[qw][c]
                        if tri:
                            # only the 128-wide diagonal band is partial
                            hi = min(off + 128, W)
                            em = emp.tile([128, 128], BF16, tag="em", name=f"em{b}_{qw}_{c}")
                            nc.gpsimd.dma_start(
                                out=em[:, :hi - off],
                                in_=emaskT[c * 128:(c + 1) * 128,
                                           qw * W + off:qw * W + hi])
                        else:
                            em = emp.tile([128, W], BF16, tag="em", name=f"em{b}_{qw}_{c}")
                            nc.gpsimd.dma_start(
                                out=em,
                                in_=emaskT[c * 128:(c + 1) * 128, qw * W:(qw + 1) * W])
                        em_tiles[c] = em
                return active, em_tiles

            def attn_window(b, qw, h, active, em_tiles, pe_dsum=False):
                """Scores -> exp -> AV accumulate -> denominator -> normalize
                for one 512-wide q window of one head. Masked (diagonal) tiles
                only compute columns >= off: columns below the first unmasked
                one are exactly zero and contribute nothing downstream."""
                qc = b * S + qw * W
                att = psC.tile([128, W], F32, tag="att", name=f"att{b}_{h}_{qw}")
                dsm = psC.tile([128, W], F32, tag="dsum", bufs=1, name=f"dsm{b}_{h}_{qw}")
                acc = None
                if not pe_dsum:
                    acc = accp.tile([128, W], BF16, tag="acc", name=f"acc{b}_{h}_{qw}")
                n = len(active)
                for ci, c in enumerate(active):
                    kind, off, tri = cls[qw][c]
                    if kind != MASKED:
                        off = 0
                    wd = W - off
                    sp = psS.tile([128, W], F32, tag="s", name=f"sc{b}_{h}_{qw}_{c}")
                    kc = b * S + c * 128
                    nc.tensor.matmul(sp[:, off:], lhsT=k_sb[h][:, kc:kc + 128],
                                     rhs=q_sb[h][:, qc + off:qc + W],
                                     start=True, stop=True)
                    pb = prp.tile([128, W], BF16, tag="probs", name=f"pb{b}_{h}_{qw}_{c}")
                    nc.scalar.activation(pb[:, off:], sp[:, off:],
                                         mybir.ActivationFunctionType.Exp,
                                         scale=RSQRT_HD)
                    if kind == MASKED and off < W:
                        if tri:
                            hi = min(off + 128, W)
                            nc.vector.tensor_mul(pb[:, off:hi], pb[:, off:hi],
                                                 em_tiles[c][:, :hi - off])
                        else:
                            nc.vector.tensor_mul(pb[:, off:], pb[:, off:],
                                                 em_tiles[c][:, off:])
                    g = b * NKC + c
                    o0 = g * DHC + h * 128
                    nc.tensor.matmul(att[:, off:], lhsT=vT_sb[:, o0:o0 + 128],
                                     rhs=pb[:, off:],
                                     start=(ci == 0), stop=(ci == n - 1))
                    if pe_dsum:
                        nc.tensor.matmul(dsm[:, off:], lhsT=ones, rhs=pb[:, off:],
                                         start=(ci == 0), stop=(ci == n - 1))
                    elif ci == 0:
                        nc.vector.tensor_copy(acc, pb)
                    else:
                        nc.vector.tensor_add(acc[:, off:], acc[:, off:], pb[:, off:])
                if not pe_dsum:
                    nc.tensor.matmul(dsm, lhsT=ones, rhs=acc, start=True, stop=True)
                rc = smp.tile([128, W], F32, tag="recip", name=f"rc{b}_{h}_{qw}")
                with tc.high_priority():
                    nc.vector.reciprocal(rc, dsm)
                    nc.vector.tensor_mul(a_sb[h][:, qc:qc + W], att, rc)

            def outproj_block(st, split_dma=False):
                """Out-projection for one 128-row seq block (both heads)."""
                for dgg in range(2):
                    ops = [psQ.tile([128, W], F32, tag="q", name=f"o{st}_{dgg}_{d2}")
                           for d2 in range(2)]
                    for h in range(HC):
                        for d2 in range(2):
                            dg = dgg * 2 + d2
                            nc.tensor.matmul(
                                ops[d2], lhsT=a_sb[h][:, st * 128:(st + 1) * 128],
                                rhs=woT_sb[h][:, dg * W:(dg + 1) * W],
                                start=(h == 0), stop=(h == HC - 1))
                    for d2 in range(2):
                        dg = dgg * 2 + d2
                        ob = outp.tile([128, W], BF16, tag="ob", name=f"ob{st}_{dg}")
                        with tc.high_priority():
                            if (st + dg) % 2 == 0:
                                nc.scalar.copy(ob, ops[d2])
                            else:
                                nc.vector.tensor_copy(ob, ops[d2])
                        dst = out_d[st * 128:(st + 1) * 128, dg * W:(dg + 1) * W]
                        if split_dma:
                            # split the tail DMAs across queues so the last
                            # store isn't a single serialized transfer
                            nc.sync.dma_start(out=dst[0:64, :], in_=ob[0:64, :])
                            nc.sync.dma_start(out=dst[64:128, :], in_=ob[64:128, :])
                        else:
                            nc.sync.dma_start(out=dst, in_=ob)

            # ---- emission schedule ----
            # QKV for batch 0 back to back; then batch-1 QKV groups interleaved
            # with batch-0 attention windows (fills exp-bound PE bubbles); each
            # window pair is chased by its out-projection blocks.
            for sg in range(2):
                xts = xts_cur
                xts_cur = load_xt_group(sg + 1)
                if sg == 0:
                    nc.scalar.dma_start(out=trig_sb, in_=trig[:, :])
                else:
                    for h in range(HC):
                        nc.scalar.dma_start(out=woT_sb[h],
                                            in_=woT[h * 128:(h + 1) * 128, :])
                for ti in range(3):
                    for dh in range(2):
                        qkv_group(sg, ti, dh, xts)

            # batch-1 QKV (12 groups) interleaved with batch-0 attention
            # (8 windows + em prefetch + outproj chasers)
            b0_units = []
            for qw in range(NQW):
                active, em_tiles = attn_window_em(0, qw)
                for h in range(HC):
                    b0_units.append(("w", 0, qw, h, active, em_tiles))
                if qw >= 1:
                    b0_units.append(("o", 0, qw - 1))
            qkv_units = []
            for sg in range(2, 4):
                qkv_units.append(("x", sg))
                for ti in range(3):
                    for dh in range(2):
                        qkv_units.append(("g", sg, ti, dh))
            qi = wi2 = 0
            sched = []
            while qi < len(qkv_units) or wi2 < len(b0_units):
                if qi < len(qkv_units):
                    sched.append(qkv_units[qi]); qi += 1
                    if qi < len(qkv_units) and qkv_units[qi][0] == "x":
                        sched.append(qkv_units[qi]); qi += 1
                if wi2 < len(b0_units):
                    sched.append(b0_units[wi2]); wi2 += 1
            for u in sched:
                if u[0] == "x":
                    xts = xts_cur
                    if u[1] + 1 < NSG:
                        xts_cur = load_xt_group(u[1] + 1)
                elif u[0] == "g":
                    qkv_group(u[1], u[2], u[3], xts)
                elif u[0] == "w":
                    attn_window(u[1], u[2], u[3], u[4], u[5])
                else:
                    for st in range(u[1] * 16 + u[2] * 4, u[1] * 16 + u[2] * 4 + 4):
                        outproj_block(st)

            # batch-1 attention; outproj chases one window behind so the
            # a_sb normalize latency hides under the next window's work
            for qw in range(NQW):
                active, em_tiles = attn_window_em(1, qw)
                for h in range(HC):
                    attn_window(1, qw, h, active, em_tiles)
                if qw == 0:
                    for st in range(12, 16):          # b0 qw3 blocks
                        outproj_block(st)
                else:
                    for st in range(16 + (qw - 1) * 4, 16 + qw * 4):
                        outproj_block(st)
            for st in range(28, 32):
                outproj_block(st, split_dma=True)
    _split_multi_waits(nc)
    return nc


def _prepare(x, freqs_cos, freqs_sin, mask, wq, wk, wv, wo):
    x = np.asarray(x, dtype=np.float32)
    wq = np.asarray(wq, dtype=np.float32)
    wk = np.asarray(wk, dtype=np.float32)
    wv = np.asarray(wv, dtype=np.float32)
    wo = np.asarray(wo, dtype=np.float32)
    fc = np.asarray(freqs_cos, dtype=np.float32)
    fs = np.asarray(freqs_sin, dtype=np.float32)
    mask = np.asarray(mask, dtype=np.float32)

    xT = np.ascontiguousarray(x.reshape(BS, D).T).astype(NPBF16)

    cosT = fc.T                      # [64, S]
    sinT = fs.T
    cos_dup = np.vstack([cosT, cosT])
    sin_sgn = np.vstack([-sinT, sinT])
    trig = np.ascontiguousarray(np.hstack([cos_dup, sin_sgn])).astype(NPBF16)

    em = np.exp(mask).T              # [k, q]; exp(-inf)=0, exp(0)=1
    emaskT = np.ascontiguousarray(em).astype(NPBF16)
    cls = []
    for qw in range(NQW):
        row = []
        for c in range(NKC):
            t = emaskT[c * 128:(c + 1) * 128, qw * W:(qw + 1) * W]
            if not t.any():
                row.append((SKIP, 0, False))
            elif (t == NPBF16(1.0)).all():
                row.append((FREE, 0, False))
            else:
                colnz = (np.asarray(t, dtype=np.float32) != 0).any(axis=0)
                off = int(np.argmax(colnz))  # first column with any valid entry
                hi = min(off + 128, W)
                tri = bool((t[:, hi:] == NPBF16(1.0)).all())
                row.append((MASKED, off, tri))
        cls.append(tuple(row))
    cls_key = tuple(cls)

    def chunk_major(wT):
        # [D, DHC] -> [128, NDIN*DHC] where [p, di*DHC+c] = wT[di*128+p, c]
        return np.ascontiguousarray(
            wT.reshape(NDIN, 128, DHC).transpose(1, 0, 2).reshape(128, NDIN * DHC)
        ).astype(NPBF16)

    # deinterleave perm: even dims then odd dims, per head
    ridx = np.concatenate([np.arange(0, HD, 2), np.arange(1, HD, 2)])
    in_maps = []
    for core in range(NCORES):
        heads = [core * HC + h for h in range(HC)]
        qk_rows = np.concatenate([g * HD + ridx for g in heads])
        v_rows = np.concatenate([np.arange(g * HD, (g + 1) * HD) for g in heads])
        m = {
            "xT": xT,
            "wqT": chunk_major(wq[qk_rows].T),
            "wkT": chunk_major(wk[qk_rows].T),
            "wvT": chunk_major(wv[v_rows].T),
            "woT": np.ascontiguousarray(wo[:, v_rows].T).astype(NPBF16),
            "trig": trig,
            "emaskT": emaskT,
        }
        in_maps.append(m)
    return in_maps, cls_key


def kernel(x, start_pos, freqs_cos, freqs_sin, mask, wq, wk, wv, wo):
    in_maps, cls_key = _prepare(x, freqs_cos, freqs_sin, mask, wq, wk, wv, wo)
    nc = _PROGRAM_CACHE.get(cls_key)
    if nc is None:
        nc = _build(cls_key)
        _PROGRAM_CACHE[cls_key] = nc
    res = run_bass_kernel_spmd(
        nc, in_maps, list(range(NCORES)),
        trace=bool(os.environ.get("KERNEL_TRACE")),
        tmpdir=os.environ.get("KERNEL_TRACE_DIR") or None)
    LAST_RUN[0] = res
    out = np.zeros([BS, D], np.float32)
    for r in res.results:
        out += np.asarray(r["out"], dtype=np.float32)
    return out.reshape(B, S, D)
